# revision 11
# baseline (speedup 1.0000x reference)
"""Self-contained Trainium2 Bass kernel: UR5 DH forward kinematics (position).

kernel(joint_angles [1048576,6] f32, dh_params [6,4] f32) -> [1048576,3] f32

Sharding: pure data parallel, batch split across 8 NeuronCores. The host
performs input canonicalization (as the earlier baseline did with its
transpose/f16-cast/permute): it forms the angle sums q23 = q2+q3 and
q234 = q2+q3+q4, wraps all five used angle streams into [-pi, pi] (the FK is
2pi-periodic in each stream, so these are value-identical inputs), and uploads
W = [w1, w2, w23, w5, w234] as [5, b_core] f16. The device computes all
transcendentals and the position assembly; outputs come back [3, b_core] f16
as [px, py, Y] and the host adds the constant d1 to the z column and upcasts.

Device pipeline (per chunk; chunks (160,288,288,288) cols of 128 partitions,
emitted stage-major so DVE/ACT/DMA overlap across chunks):
  DVE  A = W & 0x7FFF                                   (|w|)
  ACT  S-op  Sin(W[0:5])        -> SC planes 0,2,4,6,8  [s1,s2,s23,s5,s234]
       C-op  Sin(pi/2 - A[0:4]) -> SC planes 1,3,5,7    [c1,c2,c23,c5]
       NC-op Sin(A[4] - pi/2)   -> SC plane 9           -c234
  DVE  plane 10 = -s5; then the closed form (identical algebra to chaining
       [NS/G1 run on ACT for the last two chunks and D2 = [v2,-v2] on ACT
       for all but the first, filling ACT's idle tail while DVE drains]
       the six 4x4 DH transforms):
         [Y,X] = a2[s2,c2] + a3[s23,c23] + d5[-c234,s234] - d6[s5*s234,s5*c234]
         v2 = d4 + d6*c5 ; px = c1*X + s1*v2 ; py = s1*X - c1*v2 ; pz = Y + d1
       with merged ops: U = [s5,-s5](stride-4 pair view) * [s234,-c234];
       R1 = [c1,s1](reversed pair) * X-broadcast(stride-0 AP).
Verified rel err 1.7e-3 vs the f64 reference chain (gate 2e-2).
"""
import math

import numpy as np

import concourse.bass as bass
import concourse.mybir as mybir
from concourse.tile import TileContext
from concourse import tile as _tile
from concourse import bass_utils

F32 = mybir.dt.float32
F16 = mybir.dt.float16
U16 = mybir.dt.uint16
PI = math.pi
HALF_PI = 0.5 * math.pi

P = 128
N_CORES = 8
CHUNKS = (160, 288, 288, 288)

# ---------------------------------------------------------------------------
# This container's walrus build encodes at most ONE semaphore wait per
# instruction. Two fixups: (a) the TileContext exit drain gets one wait per
# DMA-sem lane -> split across several drains; (b) Tile's scheduler can attach
# two waits to a compute instruction -> hoist extras onto standalone
# same-engine EventSemaphore carriers placed just before it.
# ---------------------------------------------------------------------------


def _patched_drain_and_barrier(self, tick_clock, wait_clock):
    nc = self.nc
    carrier = nc.sync.drain()
    wait_clock.add_sem_waits(
        carrier.ins, _tile.ScopedClock({None: tick_clock.global_clock})
    )
    si = carrier.ins.sync_info
    if si is not None and len(si.on_wait) > 1:
        waits = list(si.on_wait)
        carrier.ins.sync_info = mybir.SyncInfo(on_wait=[waits[0]], on_update=[])
        for w in waits[1:]:
            extra = nc.sync.drain()
            extra.ins.sync_info = mybir.SyncInfo(on_wait=[w], on_update=[])

    nc.all_engine_barrier()
    assert self.sems is not None
    popped = nc._tile_sem_poison_stack.pop()
    assert popped is self._sem_poison
    nc.clear_and_free_semaphores(list(self.sems.allocated().values()))
    nc.all_engine_barrier()


_tile.TileContext._drain_and_barrier = _patched_drain_and_barrier

_split_counter = [0]


def _split_multi_waits(nc):
    for func in nc.m.functions:
        for bb in func.blocks:
            insts = bb.instructions
            new_list = []
            changed = False
            for inst in insts:
                si = inst.sync_info
                waits = list(si.on_wait) if si is not None else []
                if len(waits) > 1:
                    changed = True
                    for w in waits[:-1]:
                        _split_counter[0] += 1
                        carrier = mybir.InstEventSemaphore(
                            name=f"WSPLIT-{_split_counter[0]}", ins=[], outs=[])
                        carrier.engine = inst.engine
                        carrier.sync_info = mybir.SyncInfo(on_wait=[w], on_update=[])
                        new_list.append(carrier)
                    inst.sync_info = mybir.SyncInfo(
                        on_wait=[waits[-1]], on_update=list(si.on_update))
                new_list.append(inst)
            if changed:
                bb.instructions = new_list


def _bcast_pair(ap):
    """[P,1,n] view -> [P,2,n] with middle stride 0 (broadcast operand)."""
    ap2 = ap.copy()
    a = [list(d) for d in ap2.ap]
    assert a[-2][1] == 1
    a[-2] = [0, 2]
    ap2.ap = type(ap2.ap)(a)
    return ap2


def _build_fk_nc(b_core: int, dh: np.ndarray, repeat: int = 1,
                 chunks=CHUNKS):
    """SC flat planes [P, 11, n]:
    0 s1, 1 c1, 2 s2, 3 c2, 4 s23, 5 c23, 6 s5, 7 c5, 8 s234, 9 -c234, 10 -s5
    Out tile T = [px, py, Y, X]; DMA sends T[:, 0:3]."""
    dh = np.asarray(dh, dtype=np.float64)
    a2 = float(dh[1, 2]); a3 = float(dh[2, 2])
    d4 = float(dh[3, 1]); d5 = float(dh[4, 1]); d6 = float(dh[5, 1])

    assert b_core % P == 0
    ncol = b_core // P
    assert sum(chunks) == ncol

    nc = bass.Bass("TRN2")
    ja = nc.dram_tensor("ja", [5, b_core], F16, kind="ExternalInput")
    out = nc.dram_tensor("pos", [3, b_core], F16, kind="ExternalOutput")

    bias = {}
    for k, v in {"halfpi": HALF_PI, "neghalfpi": -HALF_PI,
                 "d4p": float(dh[3, 1]), "d4n": -float(dh[3, 1])}.items():
        t = nc.alloc_sbuf_tensor(k, [P, 1], F32)
        nc.gpsimd.memset(t.ap(), v)
        bias[k] = t.ap()
    nc.all_engine_barrier()

    ja3 = ja[:].rearrange("c (p m) -> p c m", p=P)    # [P, 5, ncol]
    out3 = out[:].rearrange("c (p m) -> p c m", p=P)  # [P, 3, ncol]

    Sin = mybir.ActivationFunctionType.Sin
    Ident = mybir.ActivationFunctionType.Identity
    ADD = mybir.AluOpType.add
    MULT = mybir.AluOpType.mult
    BAND = mybir.AluOpType.bitwise_and

    import contextlib

    n_chunks = len(chunks)
    offs = [sum(chunks[:i]) for i in range(n_chunks)]

    with TileContext(nc) as tc:
        with tc.tile_pool(name="fk", bufs=1) as pool:
            with (tc.For_i(0, repeat) if repeat > 1
                  else contextlib.nullcontext()):
                W, A, SC = [], [], []
                for ci, n in enumerate(chunks):
                    sl = slice(offs[ci], offs[ci] + n)
                    Wc = pool.tile([P, 5, n], F16, tag=f"W{ci}")
                    nc.sync.dma_start(out=Wc[:], in_=ja3[:, :, sl])
                    W.append(Wc)
                for ci, n in enumerate(chunks):
                    Ac = pool.tile([P, 5, n], F16, tag=f"A{ci}")
                    nc.vector.tensor_scalar(Ac[:].bitcast(U16),
                                            W[ci][:].bitcast(U16),
                                            0x7FFF, None, BAND)
                    A.append(Ac)
                ACT_FROM = n_chunks - 2
                D2_FROM = 1
                for ci, n in enumerate(chunks):
                    SCc = pool.tile([P, 11, n], F16, tag=f"SC{ci}")
                    nc.scalar.activation(SCc[:, 0:10:2], W[ci][:], Sin,
                                         scale=1.0)
                    nc.scalar.activation(SCc[:, 1:8:2], A[ci][:, 0:4], Sin,
                                         bias=bias["halfpi"], scale=-1.0)
                    nc.scalar.activation(SCc[:, 9], A[ci][:, 4], Sin,
                                         bias=bias["neghalfpi"], scale=1.0)
                    if ci >= ACT_FROM:
                        nc.scalar.activation(SCc[:, 10], W[ci][:, 3], Sin,
                                             scale=-1.0)
                    SC.append(SCc)
                G1H, D2H = {}, {}
                for ci, n in enumerate(chunks):
                    if ci >= ACT_FROM:
                        G1c = pool.tile([P, 2, n], F16, tag=f"G1{ci}")
                        nc.scalar.activation(G1c[:], SC[ci][:, 2:4], Ident,
                                             scale=a2)
                        G1H[ci] = G1c
                    if ci >= D2_FROM:
                        D2c = pool.tile([P, 2, n], F16, tag=f"D2{ci}")
                        nc.scalar.activation(D2c[:, 0], SC[ci][:, 7], Ident,
                                             bias=bias["d4p"], scale=d6)
                        nc.scalar.activation(D2c[:, 1], SC[ci][:, 7], Ident,
                                             bias=bias["d4n"], scale=-d6)
                        D2H[ci] = D2c
                for ci, n in enumerate(chunks):
                    sl = slice(offs[ci], offs[ci] + n)
                    SCc = SC[ci]
                    if ci not in G1H:
                        nc.vector.tensor_scalar(SCc[:, 10], SCc[:, 6], -1.0,
                                                None, MULT)
                    if ci in D2H:
                        D2 = D2H[ci]
                    else:
                        D2 = pool.tile([P, 2, n], F16, tag=f"D2{ci}")
                    U = pool.tile([P, 2, n], F16, tag=f"U{ci}")
                    R1 = pool.tile([P, 2, n], F16, tag=f"R1{ci}")
                    R2 = pool.tile([P, 2, n], F16, tag=f"R2{ci}")
                    T = pool.tile([P, 4, n], F16, tag=f"T{ci}")
                    YX = T[:, 2:4]
                    # U = [s5, -s5] * [s234, -c234] = [s5*s234, s5*c234]
                    nc.vector.tensor_tensor(U[:], SCc[:, 6:11:4],
                                            SCc[:, 8:10], MULT)
                    if ci in G1H:
                        G1 = G1H[ci]
                    else:
                        G1 = pool.tile([P, 2, n], F16, tag=f"G1{ci}")
                        nc.vector.tensor_scalar(G1[:], SCc[:, 2:4], a2, None,
                                                MULT)
                    G2 = pool.tile([P, 2, n], F16, tag=f"G2{ci}")
                    YB = pool.tile([P, 2, n], F16, tag=f"YB{ci}")
                    nc.vector.scalar_tensor_tensor(G2[:], SCc[:, 4:6], a3,
                                                   G1[:], MULT, ADD)
                    nc.vector.scalar_tensor_tensor(YB[:], SCc[:, 9:7:-1],
                                                   d5, G2[:], MULT, ADD)
                    nc.vector.scalar_tensor_tensor(YX[:], U[:], -d6,
                                                   YB[:], MULT, ADD)
                    if ci not in D2H:
                        nc.vector.tensor_scalar(D2[:, 0], SCc[:, 7], d6, d4,
                                                MULT, ADD)
                        nc.vector.tensor_scalar(D2[:, 1], SCc[:, 7], -d6, -d4,
                                                MULT, ADD)
                    # R1 = [c1, s1] * X ; R2 = [s1, c1] * [v2, -v2]
                    nc.vector.tensor_tensor(R1[:], SCc[:, 1::-1],
                                            _bcast_pair(T[:, 3:4]), MULT)
                    nc.vector.tensor_tensor(R2[:], SCc[:, 0:2], D2[:], MULT)
                    nc.vector.tensor_tensor(T[:, 0:2], R1[:], R2[:], ADD)
                    nc.sync.dma_start(out=out3[:, :, sl], in_=T[:, 0:3])

    _split_multi_waits(nc)
    return nc


_NC_CACHE: dict[tuple, object] = {}


def make_in_maps(ja: np.ndarray) -> list[dict]:
    """[B, 6] f32 raw angles -> per-core {'ja': [5, b] f16 wrapped}."""
    ja64 = np.asarray(ja, dtype=np.float64)
    q23 = ja64[:, 1] + ja64[:, 2]
    W = np.stack([ja64[:, 0], ja64[:, 1], q23, ja64[:, 4],
                  q23 + ja64[:, 3]], axis=0)  # [w1, w2, w23, w5, w234]
    W = W - 2 * PI * np.round(W / (2 * PI))
    W = W.astype(np.float16)
    b = ja.shape[0] // N_CORES
    return [{"ja": np.ascontiguousarray(W[:, i * b:(i + 1) * b])}
            for i in range(N_CORES)]


def kernel(joint_angles: np.ndarray, dh_params: np.ndarray) -> np.ndarray:
    ja = np.asarray(joint_angles, dtype=np.float32)
    dh = np.asarray(dh_params, dtype=np.float64)
    B = ja.shape[0]
    assert B % N_CORES == 0
    b_core = B // N_CORES

    key = (b_core, dh.tobytes())
    nc = _NC_CACHE.get(key)
    if nc is None:
        nc = _build_fk_nc(b_core, dh)
        _NC_CACHE[key] = nc

    in_maps = make_in_maps(ja)
    res = bass_utils.run_bass_kernel_spmd(nc, in_maps,
                                          core_ids=list(range(N_CORES)))
    out = np.concatenate([r["pos"].T.astype(np.float32)
                          for r in res.results], axis=0)
    out[:, 2] += np.float32(dh[0, 1])  # pz = Y + d1
    return out
